# revision 26
# baseline (speedup 1.0000x reference)
"""Trainium2 Bass kernel for nn_MetadataTapas (segment_reduce).

Strategy (pure data-parallel over batch, 4 rows per core on 8 cores):
  - segment-mean as a one-hot matmul on the TensorEngine:
      sums[f, h] = sum_s (col_ids[s] == f+1) * emb[s, h]
  - All seven heads are linear before log_softmax, so:
      head_out = diag(1/cnt) @ OneHot^T @ Emb @ W + b
    fe = sums * inv_cnt, transposed on the PE, then one fused matmul
    against the concatenated head weights (28 cols incl. the pair
    head's two 768-col halves u1/u2).
  - Pair gathers become selection matmuls: SelT[f, p] = (idx[p] == f)
    built on the DVE from a PE rank-1 broadcast, used as matmul weights
    against u = [u1 | u2].
  - log_softmax per head: reduce_max(negate) -> ACT exp(bias=-m,
    accum_out=sum) -> ACT ln -> fused tensor_scalar (x + (-m)) - ls.
  - Every DMA is contiguous: emb is one 3MB DMA per row, the small
    per-row inputs are host-packed into two tiny tensors, and all
    outputs leave as one [28, 128] block per row that the host
    reslices into the seven reference outputs.
"""

import os
import numpy as np
from contextlib import ExitStack

import concourse.bass as bass
import concourse.bacc as bacc
import concourse.mybir as mybir
import concourse.tile as tile

B, S, H, F, P, NTYPE = 32, 1024, 768, 128, 256, 7
NCORES = 8
RPC = B // NCORES          # batch rows per core
NCHUNK = S // 128          # 8 token chunks per row
KH = H // 128              # 6 contraction tiles over H
NH = 28                    # packed head cols: msr2 dim2 msrs2 key2 agg9 type7 u1_2 u2_2
METAW = 12                 # per-row meta cols: cid_f(8) inv(1) pad(3)

F32 = mybir.dt.float32
F32R = mybir.dt.float32r
I32 = mybir.dt.int32

HEAD_SLICES = [(0, 2), (2, 4), (4, 6), (6, 8), (8, 17), (17, 24)]

# Big matmuls in float32r: full-rate fp32 on the PE for moving dim >= 256.
# The BIR verifier requires fp32r matmul operands to be *produced* as
# float32r, so the whole emb/one-hot/idx path is typed float32r.
SEG_MM_F32R = os.environ.get("SEG_MM_F32R", "1") == "1"
EMB_DT = F32R if SEG_MM_F32R else F32


def _softmax_pre(nc, ps_ap, e_ap, negm_ap, ssum_ap):
    """reduce_max(negate) then exp(x - m) with accumulated sum.

    All Exp ops are batched before the single Ln per row so the ACT
    engine loads each activation table at most twice per row (table
    reloads were the dominant cost when Exp/Ln alternated)."""
    nc.vector.tensor_reduce(
        negm_ap, ps_ap, axis=mybir.AxisListType.X, op=mybir.AluOpType.max, negate=True
    )
    nc.scalar.activation(
        e_ap, ps_ap, mybir.ActivationFunctionType.Exp,
        bias=negm_ap, scale=1.0, accum_out=ssum_ap,
    )


def _build_body(ctx, tc, aps):
    nc = tc.nc
    emb, meta, pidxf = aps["emb"], aps["meta"], aps["pidxf"]
    o_all = aps["o_all"]

    const = ctx.enter_context(tc.tile_pool(name="const", bufs=1))
    embp = ctx.enter_context(tc.tile_pool(name="embp", bufs=8))
    ohp = ctx.enter_context(tc.tile_pool(name="ohp", bufs=2))
    rowp = ctx.enter_context(tc.tile_pool(name="rowp", bufs=2))
    stat = ctx.enter_context(tc.tile_pool(name="stat", bufs=4))
    psA = ctx.enter_context(tc.tile_pool(name="psA", bufs=2, space="PSUM"))
    psT = ctx.enter_context(tc.tile_pool(name="psT", bufs=2, space="PSUM"))
    psS = ctx.enter_context(tc.tile_pool(name="psS", bufs=2, space="PSUM"))

    # constants
    w_t = const.tile([128, KH * NH], F32, tag="w_t")
    nc.sync.dma_start(w_t[:], aps["w_all"])
    b_t = const.tile([1, NH], F32, tag="b_t")
    nc.sync.dma_start(b_t[:], aps["b_row"])
    ir_t = const.tile([128, 128], F32, tag="ir_t")
    nc.sync.dma_start(ir_t[:], aps["iota_row"])
    ic_t = const.tile([128, 1], F32, tag="ic_t")
    nc.sync.dma_start(ic_t[:], aps["iota_col"])
    id_t = const.tile([128, 128], F32, tag="id_t")
    nc.sync.dma_start(id_t[:], aps["ident"])
    on_t = const.tile([1, 128], F32, tag="on_t")
    nc.sync.dma_start(on_t[:], aps["ones128"])
    onr_t = const.tile([1, 128], EMB_DT, tag="onr_t")
    nc.sync.dma_start(onr_t[:], aps["ones128r"])

    for r in range(RPC):
        # --- per-row small inputs: one contiguous DMA each
        mt = rowp.tile([128, METAW], F32, tag="mt")
        nc.sync.dma_start(mt[:], meta[r])
        cid_f = mt[:, 0:NCHUNK]
        inv = mt[:, NCHUNK : NCHUNK + 1]

        pxf = rowp.tile([1, 2 * P], EMB_DT, tag="pxf")
        nc.sync.dma_start(pxf[:], pidxf[r])

        # broadcast idx row to all partitions via rank-1 matmul: ones^T @ idx
        idxb = psS.tile([128, 2 * P], F32, tag="sps")
        nc.tensor.matmul(idxb[:], onr_t[:], pxf[:], start=True, stop=True)
        selT = rowp.tile([128, 2 * P], F32, tag="selT")
        nc.vector.tensor_scalar(
            out=selT[:], in0=idxb[:], scalar1=ic_t[:, 0:1], scalar2=None,
            op0=mybir.AluOpType.is_equal,
        )

        # --- all 8 one-hot blocks in one DVE op via step-0 broadcasts:
        # oh_row[p, c, j] = (cid[p, c] == j + 1)
        oh_row = ohp.tile([128, NCHUNK * 128], EMB_DT, tag="oh")
        nc.vector.tensor_tensor(
            out=oh_row[:].rearrange("p (c j) -> p c j", c=NCHUNK),
            in0=cid_f.unsqueeze(2).broadcast_to([128, NCHUNK, 128]),
            in1=ir_t[:].unsqueeze(1).broadcast_to([128, NCHUNK, 128]),
            op=mybir.AluOpType.is_equal,
        )

        # --- embeddings: 4 pipelined ~768KB DMAs per row (2 chunks each) so
        # the PE can start on chunk 0 after ~1/4 of the row has landed
        ets = []
        for g in range(4):
            et = embp.tile([128, 2 * H], EMB_DT, tag="et")
            nc.sync.dma_start(
                et[:],
                emb[r, 2 * g * 128 : 2 * (g + 1) * 128, :].rearrange(
                    "(c p) h -> p c h", p=128
                ),
            )
            ets.append(et)

        # --- segment sums via one-hot matmul
        ps = psA.tile([128, 768], F32, tag="ps")
        for c in range(NCHUNK):
            oh = oh_row[:, bass.ts(c, 128)]
            et = ets[c // 2]
            off = (c % 2) * H
            nc.tensor.matmul(ps[:, 0:512], oh, et[:, off : off + 512],
                             start=(c == 0), stop=(c == NCHUNK - 1))
            nc.tensor.matmul(ps[:, 512:768], oh, et[:, off + 512 : off + 768],
                             start=(c == 0), stop=(c == NCHUNK - 1))

        # --- field embedding = sums * (1/max(cnt,1))  (inv_cnt host-derived)
        fe = rowp.tile([128, 768], F32, tag="fe")
        nc.vector.tensor_scalar_mul(fe[:], ps[:, 0:768], inv)

        # --- transpose fe -> feT (h on partitions)
        feT = rowp.tile([128, 768], F32, tag="feT")
        for k in range(KH):
            pt = psT.tile([128, 128], F32, tag="pt")
            nc.tensor.transpose(pt[:], fe[:, bass.ts(k, 128)], id_t[:])
            nc.vector.tensor_copy(feT[:, bass.ts(k, 128)], pt[:])

        # --- all heads in one accumulated matmul; bias via rank-1 matmul
        ph = psS.tile([128, NH], F32, tag="sps")
        for k in range(KH):
            nc.tensor.matmul(
                ph[:], feT[:, bass.ts(k, 128)], w_t[:, bass.ts(k, NH)],
                start=(k == 0), stop=False,
            )
        nc.tensor.matmul(ph[:], on_t[:], b_t[:], start=False, stop=True)

        # pair-head rhs must live in SBUF
        u = rowp.tile([128, 4], F32, tag="u")
        nc.vector.tensor_copy(u[:], ph[:, 24:28])

        # --- log_softmax, batched: 8x (max, exp+accum), 1x ln, 8x fused sub
        negms = stat.tile([128, 8], F32, tag="negms")
        ssums = stat.tile([128, 8], F32, tag="ssums")
        lss = stat.tile([128, 8], F32, tag="lss")
        e_all = stat.tile([128, NH], F32, tag="e_all")
        ho = rowp.tile([128, NH], F32, tag="ho")
        for i, (a, b) in enumerate(HEAD_SLICES):
            _softmax_pre(nc, ph[:, a:b], e_all[:, a:b],
                         negms[:, i : i + 1], ssums[:, i : i + 1])

        # --- pair head: gather-as-matmul
        pp = psS.tile([128, 4], F32, tag="sps")
        for h in range(2):
            nc.tensor.matmul(
                pp[:, 2 * h : 2 * h + 2], selT[:, bass.ts(h, 128)], u[:, 0:2],
                start=True, stop=False,
            )
            nc.tensor.matmul(
                pp[:, 2 * h : 2 * h + 2], selT[:, 256 + h * 128 : 256 + (h + 1) * 128],
                u[:, 2:4], start=False, stop=True,
            )
        for h in range(2):
            _softmax_pre(nc, pp[:, 2 * h : 2 * h + 2], e_all[:, 24 + 2 * h : 26 + 2 * h],
                         negms[:, 6 + h : 7 + h], ssums[:, 6 + h : 7 + h])

        nc.scalar.activation(lss[:], ssums[:], mybir.ActivationFunctionType.Ln)
        for i, (a, b) in enumerate(HEAD_SLICES):
            nc.vector.tensor_scalar(
                out=ho[:, a:b], in0=ph[:, a:b],
                scalar1=negms[:, i : i + 1], scalar2=lss[:, i : i + 1],
                op0=mybir.AluOpType.add, op1=mybir.AluOpType.subtract,
            )
        for h in range(2):
            nc.vector.tensor_scalar(
                out=ho[:, 24 + 2 * h : 26 + 2 * h], in0=pp[:, 2 * h : 2 * h + 2],
                scalar1=negms[:, 6 + h : 7 + h], scalar2=lss[:, 6 + h : 7 + h],
                op0=mybir.AluOpType.add, op1=mybir.AluOpType.subtract,
            )

        # --- transpose outputs and ship one contiguous block per row
        po = psT.tile([NH, 128], F32, tag="pt")
        nc.tensor.transpose(po[:], ho[:], id_t[:])
        oT = rowp.tile([NH, 128], F32, tag="oT")
        nc.vector.tensor_copy(oT[:], po[:])
        nc.sync.dma_start(o_all[r], oT[:])


def build_program():
    nc = bacc.Bacc(trn_type="TRN2", target_bir_lowering=False, debug=False)
    aps = {}
    aps["emb"] = nc.dram_tensor("emb", [RPC, S, H], EMB_DT, kind="ExternalInput").ap()
    aps["meta"] = nc.dram_tensor("meta", [RPC, 128, METAW], F32, kind="ExternalInput").ap()
    aps["pidxf"] = nc.dram_tensor("pidxf", [RPC, 1, 2 * P], EMB_DT, kind="ExternalInput").ap()
    aps["w_all"] = nc.dram_tensor("w_all", [128, KH * NH], F32, kind="ExternalInput").ap()
    aps["b_row"] = nc.dram_tensor("b_row", [1, NH], F32, kind="ExternalInput").ap()
    aps["iota_row"] = nc.dram_tensor("iota_row", [128, 128], F32, kind="ExternalInput").ap()
    aps["iota_col"] = nc.dram_tensor("iota_col", [128, 1], F32, kind="ExternalInput").ap()
    aps["ident"] = nc.dram_tensor("ident", [128, 128], F32, kind="ExternalInput").ap()
    aps["ones128"] = nc.dram_tensor("ones128", [1, 128], F32, kind="ExternalInput").ap()
    aps["ones128r"] = nc.dram_tensor("ones128r", [1, 128], EMB_DT, kind="ExternalInput").ap()
    aps["o_all"] = nc.dram_tensor("o_all", [RPC, NH, 128], F32, kind="ExternalOutput").ap()

    with tile.TileContext(nc) as tc:
        with ExitStack() as ctx:
            _build_body(ctx, tc, aps)
    nc.compile()
    return nc


def host_constants(W_msr, b_msr, W_agg, b_agg, W_dim, b_dim, W_msrs, b_msrs,
                   W_key, b_key, W_pair, b_pair, W_type, b_type):
    f = np.float32
    W_all = np.concatenate(
        [W_msr, W_dim, W_msrs, W_key, W_agg, W_type, W_pair[:H], W_pair[H:]], axis=1
    ).astype(f)  # (768, 28)
    w_packed = np.ascontiguousarray(
        W_all.reshape(KH, 128, NH).transpose(1, 0, 2).reshape(128, KH * NH)
    )
    b_all = np.concatenate(
        [b_msr, b_dim, b_msrs, b_key, b_agg, b_type, b_pair, np.zeros(2, f)]
    ).astype(f).reshape(1, NH)
    return {
        "w_all": w_packed,
        "b_row": np.ascontiguousarray(b_all),
        "iota_row": np.tile(np.arange(1, 129, dtype=f), (128, 1)),
        "iota_col": np.arange(128, dtype=f).reshape(128, 1),
        "ident": np.eye(128, dtype=f),
        "ones128": np.ones((1, 128), dtype=f),
        "ones128r": np.ones((1, 128), dtype=f),
    }


def make_in_maps(tapas_embedding, col_ids, msr_pair_idx, consts):
    f = np.float32
    in_maps = []
    for i in range(NCORES):
        sl = slice(i * RPC, (i + 1) * RPC)
        m = dict(consts)
        m["emb"] = np.ascontiguousarray(tapas_embedding[sl], dtype=f)
        cid = np.asarray(col_ids[sl], dtype=np.int64)
        # meta[r] = [cid as f32 (p, c) | 1/max(cnt,1) | pad]
        meta = np.zeros((RPC, 128, METAW), f)
        meta[:, :, 0:NCHUNK] = (
            cid.reshape(RPC, NCHUNK, 128).transpose(0, 2, 1).astype(f)
        )
        for r in range(RPC):
            cnt = np.bincount(cid[r], minlength=F + 1)
            meta[r, :, NCHUNK] = (1.0 / np.maximum(cnt[1:], 1)).astype(f)
        m["meta"] = meta
        # pair indices, j-major, as f32
        m["pidxf"] = np.ascontiguousarray(
            np.asarray(msr_pair_idx[sl], np.int64).transpose(0, 2, 1)
            .reshape(RPC, 1, 2 * P).astype(f)
        )
        in_maps.append(m)
    return in_maps


def assemble_outputs(o_all_list):
    """o_all per core: (RPC, 28, 128) -> the seven reference outputs."""
    o = np.concatenate([np.asarray(x) for x in o_all_list], 0)  # (n, 28, 128)
    n = o.shape[0]

    def head(a, b):
        return np.ascontiguousarray(o[:, a:b, :].transpose(0, 2, 1))

    pair = np.ascontiguousarray(
        o[:, 24:28, :].reshape(n, 2, 2, 128).transpose(0, 1, 3, 2).reshape(n, P, 2)
    )
    return (head(0, 2), head(8, 17), head(4, 6), head(2, 4), head(6, 8),
            pair, head(17, 24))


_NC_CACHE = {}


def kernel(tapas_embedding, col_ids, msr_pair_idx, n_fields,
           W_msr, b_msr, W_agg, b_agg, W_dim, b_dim, W_msrs, b_msrs,
           W_key, b_key, W_pair, b_pair, W_type, b_type, **_unused):
    from concourse.bass_utils import run_bass_kernel_spmd

    assert int(n_fields) == F
    consts = host_constants(
        np.asarray(W_msr), np.asarray(b_msr), np.asarray(W_agg), np.asarray(b_agg),
        np.asarray(W_dim), np.asarray(b_dim), np.asarray(W_msrs), np.asarray(b_msrs),
        np.asarray(W_key), np.asarray(b_key), np.asarray(W_pair), np.asarray(b_pair),
        np.asarray(W_type), np.asarray(b_type),
    )
    if "nc" not in _NC_CACHE:
        _NC_CACHE["nc"] = build_program()
    nc = _NC_CACHE["nc"]
    in_maps = make_in_maps(
        np.asarray(tapas_embedding), np.asarray(col_ids), np.asarray(msr_pair_idx), consts
    )
    res = run_bass_kernel_spmd(nc, in_maps, list(range(NCORES))).results
    return assemble_outputs([res[i]["o_all"] for i in range(NCORES)])


# revision 31
# speedup vs baseline: 1.0464x; 1.0464x over previous
"""Trainium2 Bass kernel for nn_MetadataTapas (segment_reduce).

Strategy (pure data-parallel over batch, 4 rows per core on 8 cores):
  - segment-mean as a one-hot matmul on the TensorEngine:
      sums[f, h] = sum_s (col_ids[s] == f+1) * emb[s, h]
  - All seven heads are linear before log_softmax, so:
      head_out = diag(1/cnt) @ OneHot^T @ Emb @ W + b
    fe = sums * inv_cnt, transposed on the PE, then one fused matmul
    against the concatenated head weights (28 cols incl. the pair
    head's two 768-col halves u1/u2).
  - Pair gathers become selection matmuls: SelT[f, p] = (idx[p] == f)
    built on the DVE from a PE rank-1 broadcast, used as matmul weights
    against u = [u1 | u2].
  - log_softmax per head: reduce_max(negate) -> ACT exp(bias=-m,
    accum_out=sum) -> ACT ln -> fused tensor_scalar (x + (-m)) - ls.
  - Every DMA is contiguous: emb is one 3MB DMA per row, the small
    per-row inputs are host-packed into two tiny tensors, and all
    outputs leave as one [28, 128] block per row that the host
    reslices into the seven reference outputs.
"""

import os
import numpy as np
from contextlib import ExitStack

import concourse.bass as bass
import concourse.bacc as bacc
import concourse.mybir as mybir
import concourse.tile as tile

B, S, H, F, P, NTYPE = 32, 1024, 768, 128, 256, 7
NCORES = 8
RPC = B // NCORES          # batch rows per core
NCHUNK = S // 128          # 8 token chunks per row
KH = H // 128              # 6 contraction tiles over H
NH = 28                    # packed head cols: msr2 dim2 msrs2 key2 agg9 type7 u1_2 u2_2
METAW = 12                 # per-row meta cols: cid_f(8) inv(1) pad(3)

F32 = mybir.dt.float32
F32R = mybir.dt.float32r
BF16 = mybir.dt.bfloat16
I32 = mybir.dt.int32

HEAD_SLICES = [(0, 2), (2, 4), (4, 6), (6, 8), (8, 17), (17, 24)]

# Big matmuls in float32r: full-rate fp32 on the PE for moving dim >= 256.
# The BIR verifier requires fp32r matmul operands to be *produced* as
# float32r, so the whole emb/one-hot/idx path is typed float32r.
SEG_MM_F32R = os.environ.get("SEG_MM_F32R", "1") == "1"
EMB_DT = F32R if SEG_MM_F32R else F32


def _softmax_pre(nc, ps_ap, e_ap, negm_ap, ssum_ap):
    """reduce_max(negate) then exp(x - m) with accumulated sum.

    All Exp ops are batched before the single Ln per row so the ACT
    engine loads each activation table at most twice per row (table
    reloads were the dominant cost when Exp/Ln alternated)."""
    nc.vector.tensor_reduce(
        negm_ap, ps_ap, axis=mybir.AxisListType.X, op=mybir.AluOpType.max, negate=True
    )
    nc.scalar.activation(
        e_ap, ps_ap, mybir.ActivationFunctionType.Exp,
        bias=negm_ap, scale=1.0, accum_out=ssum_ap,
    )


def _build_body(ctx, tc, aps):
    nc = tc.nc
    emb, meta, pidxf = aps["emb"], aps["meta"], aps["pidxf"]
    o_all = aps["o_all"]

    const = ctx.enter_context(tc.tile_pool(name="const", bufs=1))
    embp = ctx.enter_context(tc.tile_pool(name="embp", bufs=8))
    ohp = ctx.enter_context(tc.tile_pool(name="ohp", bufs=RPC))
    rowp = ctx.enter_context(tc.tile_pool(name="rowp", bufs=2))
    prep = ctx.enter_context(tc.tile_pool(name="prep", bufs=RPC))
    stat = ctx.enter_context(tc.tile_pool(name="stat", bufs=4))
    psA = ctx.enter_context(tc.tile_pool(name="psA", bufs=2, space="PSUM"))
    psT = ctx.enter_context(tc.tile_pool(name="psT", bufs=2, space="PSUM"))
    psS = ctx.enter_context(tc.tile_pool(name="psS", bufs=2, space="PSUM"))

    # constants
    w_t = const.tile([128, KH * NH], F32, tag="w_t")
    nc.sync.dma_start(w_t[:], aps["w_all"])
    b_t = const.tile([1, NH], F32, tag="b_t")
    nc.sync.dma_start(b_t[:], aps["b_row"])
    ir_t = const.tile([128, 128], F32, tag="ir_t")
    nc.sync.dma_start(ir_t[:], aps["iota_row"])
    ic_t = const.tile([128, 1], F32, tag="ic_t")
    nc.sync.dma_start(ic_t[:], aps["iota_col"])
    id_t = const.tile([128, 128], F32, tag="id_t")
    nc.sync.dma_start(id_t[:], aps["ident"])
    on_t = const.tile([1, 128], F32, tag="on_t")
    nc.sync.dma_start(on_t[:], aps["ones128"])
    onr_t = const.tile([1, 128], EMB_DT, tag="onr_t")
    nc.sync.dma_start(onr_t[:], aps["ones128r"])

    # --- prologue: per-row small inputs, one-hots and pair selectors for ALL
    # rows up front, so the DVE never gates the next row's seg matmuls
    mts, oh_rows, selTs = [], [], []
    for r in range(RPC):
        mt = prep.tile([128, METAW], F32, tag="mt")
        nc.sync.dma_start(mt[:], meta[r])
        mts.append(mt)

        pxf = prep.tile([1, 2 * P], EMB_DT, tag="pxf")
        nc.sync.dma_start(pxf[:], pidxf[r])

        # broadcast idx row to all partitions via rank-1 matmul: ones^T @ idx
        idxb = psS.tile([128, 2 * P], F32, tag="sps")
        nc.tensor.matmul(idxb[:], onr_t[:], pxf[:], start=True, stop=True)
        selT = prep.tile([128, 2 * P], EMB_DT, tag="selT")
        nc.vector.tensor_scalar(
            out=selT[:], in0=idxb[:], scalar1=ic_t[:, 0:1], scalar2=None,
            op0=mybir.AluOpType.is_equal,
        )
        selTs.append(selT)

        # all 8 one-hot blocks in one DVE op via step-0 broadcasts:
        # oh_row[p, c, j] = (cid[p, c] == j + 1); bf16 is exact for 0/1 and
        # enables the PE fast-weight-load path
        oh_row = ohp.tile([128, NCHUNK * 128], EMB_DT, tag="oh")
        nc.vector.tensor_tensor(
            out=oh_row[:].rearrange("p (c j) -> p c j", c=NCHUNK),
            in0=mt[:, 0:NCHUNK].unsqueeze(2).broadcast_to([128, NCHUNK, 128]),
            in1=ir_t[:].unsqueeze(1).broadcast_to([128, NCHUNK, 128]),
            op=mybir.AluOpType.is_equal,
        )
        oh_rows.append(oh_row)

    for r in range(RPC):
        inv = mts[r][:, NCHUNK : NCHUNK + 1]
        oh_row = oh_rows[r]
        selT = selTs[r]

        # --- embeddings: 4 pipelined ~768KB DMAs per row (2 chunks each) so
        # the PE can start on chunk 0 after ~1/4 of the row has landed
        ets = []
        for g in range(4):
            et = embp.tile([128, 2 * H], EMB_DT, tag="et")
            nc.sync.dma_start(
                et[:],
                emb[r, 2 * g * 128 : 2 * (g + 1) * 128, :].rearrange(
                    "(c p) h -> p c h", p=128
                ),
            )
            ets.append(et)

        # --- segment sums via one-hot matmul
        ps = psA.tile([128, 768], F32, tag="ps")
        for c in range(NCHUNK):
            oh = oh_row[:, bass.ts(c, 128)]
            et = ets[c // 2]
            off = (c % 2) * H
            nc.tensor.matmul(ps[:, 0:512], oh, et[:, off : off + 512],
                             start=(c == 0), stop=(c == NCHUNK - 1))
            nc.tensor.matmul(ps[:, 512:768], oh, et[:, off + 512 : off + 768],
                             start=(c == 0), stop=(c == NCHUNK - 1))

        # --- field embedding = sums * (1/max(cnt,1))  (inv_cnt host-derived)
        fe = rowp.tile([128, 768], F32, tag="fe")
        nc.vector.tensor_scalar_mul(fe[:], ps[:, 0:768], inv)

        # --- transpose fe -> feT (h on partitions)
        feT = rowp.tile([128, 768], F32, tag="feT")
        for k in range(KH):
            pt = psT.tile([128, 128], F32, tag="pt")
            nc.tensor.transpose(pt[:], fe[:, bass.ts(k, 128)], id_t[:])
            nc.vector.tensor_copy(feT[:, bass.ts(k, 128)], pt[:])

        # --- all heads in one accumulated matmul; bias via rank-1 matmul
        ph = psS.tile([128, NH], F32, tag="sps")
        for k in range(KH):
            nc.tensor.matmul(
                ph[:], feT[:, bass.ts(k, 128)], w_t[:, bass.ts(k, NH)],
                start=(k == 0), stop=False,
            )
        nc.tensor.matmul(ph[:], on_t[:], b_t[:], start=False, stop=True)

        # pair-head rhs must live in SBUF
        u = rowp.tile([128, 4], EMB_DT, tag="u")
        nc.vector.tensor_copy(u[:], ph[:, 24:28])

        # --- log_softmax, batched: 8x (max, exp+accum), 1x ln, 8x fused sub
        negms = stat.tile([128, 8], F32, tag="negms")
        ssums = stat.tile([128, 8], F32, tag="ssums")
        lss = stat.tile([128, 8], F32, tag="lss")
        e_all = stat.tile([128, NH], F32, tag="e_all")
        ho = rowp.tile([128, NH], F32, tag="ho")
        for i, (a, b) in enumerate(HEAD_SLICES):
            _softmax_pre(nc, ph[:, a:b], e_all[:, a:b],
                         negms[:, i : i + 1], ssums[:, i : i + 1])

        # --- pair head: gather-as-matmul
        pp = psS.tile([128, 4], F32, tag="sps")
        for h in range(2):
            nc.tensor.matmul(
                pp[:, 2 * h : 2 * h + 2], selT[:, bass.ts(h, 128)], u[:, 0:2],
                start=True, stop=False,
            )
            nc.tensor.matmul(
                pp[:, 2 * h : 2 * h + 2], selT[:, 256 + h * 128 : 256 + (h + 1) * 128],
                u[:, 2:4], start=False, stop=True,
            )
        for h in range(2):
            _softmax_pre(nc, pp[:, 2 * h : 2 * h + 2], e_all[:, 24 + 2 * h : 26 + 2 * h],
                         negms[:, 6 + h : 7 + h], ssums[:, 6 + h : 7 + h])

        nc.scalar.activation(lss[:], ssums[:], mybir.ActivationFunctionType.Ln)
        for i, (a, b) in enumerate(HEAD_SLICES):
            nc.vector.tensor_scalar(
                out=ho[:, a:b], in0=ph[:, a:b],
                scalar1=negms[:, i : i + 1], scalar2=lss[:, i : i + 1],
                op0=mybir.AluOpType.add, op1=mybir.AluOpType.subtract,
            )
        for h in range(2):
            nc.vector.tensor_scalar(
                out=ho[:, 24 + 2 * h : 26 + 2 * h], in0=pp[:, 2 * h : 2 * h + 2],
                scalar1=negms[:, 6 + h : 7 + h], scalar2=lss[:, 6 + h : 7 + h],
                op0=mybir.AluOpType.add, op1=mybir.AluOpType.subtract,
            )

        # --- transpose outputs and ship one contiguous block per row
        po = psT.tile([NH, 128], F32, tag="pt")
        nc.tensor.transpose(po[:], ho[:], id_t[:])
        oT = rowp.tile([NH, 128], F32, tag="oT")
        nc.vector.tensor_copy(oT[:], po[:])
        nc.sync.dma_start(o_all[r], oT[:])


def build_program():
    nc = bacc.Bacc(trn_type="TRN2", target_bir_lowering=False, debug=False)
    aps = {}
    aps["emb"] = nc.dram_tensor("emb", [RPC, S, H], EMB_DT, kind="ExternalInput").ap()
    aps["meta"] = nc.dram_tensor("meta", [RPC, 128, METAW], F32, kind="ExternalInput").ap()
    aps["pidxf"] = nc.dram_tensor("pidxf", [RPC, 1, 2 * P], EMB_DT, kind="ExternalInput").ap()
    aps["w_all"] = nc.dram_tensor("w_all", [128, KH * NH], F32, kind="ExternalInput").ap()
    aps["b_row"] = nc.dram_tensor("b_row", [1, NH], F32, kind="ExternalInput").ap()
    aps["iota_row"] = nc.dram_tensor("iota_row", [128, 128], F32, kind="ExternalInput").ap()
    aps["iota_col"] = nc.dram_tensor("iota_col", [128, 1], F32, kind="ExternalInput").ap()
    aps["ident"] = nc.dram_tensor("ident", [128, 128], F32, kind="ExternalInput").ap()
    aps["ones128"] = nc.dram_tensor("ones128", [1, 128], F32, kind="ExternalInput").ap()
    aps["ones128r"] = nc.dram_tensor("ones128r", [1, 128], EMB_DT, kind="ExternalInput").ap()
    aps["o_all"] = nc.dram_tensor("o_all", [RPC, NH, 128], F32, kind="ExternalOutput").ap()

    with tile.TileContext(nc) as tc:
        with ExitStack() as ctx:
            _build_body(ctx, tc, aps)
    nc.compile()
    return nc


def host_constants(W_msr, b_msr, W_agg, b_agg, W_dim, b_dim, W_msrs, b_msrs,
                   W_key, b_key, W_pair, b_pair, W_type, b_type):
    f = np.float32
    W_all = np.concatenate(
        [W_msr, W_dim, W_msrs, W_key, W_agg, W_type, W_pair[:H], W_pair[H:]], axis=1
    ).astype(f)  # (768, 28)
    w_packed = np.ascontiguousarray(
        W_all.reshape(KH, 128, NH).transpose(1, 0, 2).reshape(128, KH * NH)
    )
    b_all = np.concatenate(
        [b_msr, b_dim, b_msrs, b_key, b_agg, b_type, b_pair, np.zeros(2, f)]
    ).astype(f).reshape(1, NH)
    return {
        "w_all": w_packed,
        "b_row": np.ascontiguousarray(b_all),
        "iota_row": np.tile(np.arange(1, 129, dtype=f), (128, 1)),
        "iota_col": np.arange(128, dtype=f).reshape(128, 1),
        "ident": np.eye(128, dtype=f),
        "ones128": np.ones((1, 128), dtype=f),
        "ones128r": np.ones((1, 128), dtype=f),
    }


def make_in_maps(tapas_embedding, col_ids, msr_pair_idx, consts):
    f = np.float32
    in_maps = []
    for i in range(NCORES):
        sl = slice(i * RPC, (i + 1) * RPC)
        m = dict(consts)
        m["emb"] = np.ascontiguousarray(tapas_embedding[sl], dtype=f)
        cid = np.asarray(col_ids[sl], dtype=np.int64)
        # meta[r] = [cid as f32 (p, c) | 1/max(cnt,1) | pad]
        meta = np.zeros((RPC, 128, METAW), f)
        meta[:, :, 0:NCHUNK] = (
            cid.reshape(RPC, NCHUNK, 128).transpose(0, 2, 1).astype(f)
        )
        for r in range(RPC):
            cnt = np.bincount(cid[r], minlength=F + 1)
            meta[r, :, NCHUNK] = (1.0 / np.maximum(cnt[1:], 1)).astype(f)
        m["meta"] = meta
        # pair indices, j-major, as f32
        m["pidxf"] = np.ascontiguousarray(
            np.asarray(msr_pair_idx[sl], np.int64).transpose(0, 2, 1)
            .reshape(RPC, 1, 2 * P).astype(f)
        )
        in_maps.append(m)
    return in_maps


def assemble_outputs(o_all_list):
    """o_all per core: (RPC, 28, 128) -> the seven reference outputs."""
    o = np.concatenate([np.asarray(x) for x in o_all_list], 0)  # (n, 28, 128)
    n = o.shape[0]

    def head(a, b):
        return np.ascontiguousarray(o[:, a:b, :].transpose(0, 2, 1))

    pair = np.ascontiguousarray(
        o[:, 24:28, :].reshape(n, 2, 2, 128).transpose(0, 1, 3, 2).reshape(n, P, 2)
    )
    return (head(0, 2), head(8, 17), head(4, 6), head(2, 4), head(6, 8),
            pair, head(17, 24))


_NC_CACHE = {}


def kernel(tapas_embedding, col_ids, msr_pair_idx, n_fields,
           W_msr, b_msr, W_agg, b_agg, W_dim, b_dim, W_msrs, b_msrs,
           W_key, b_key, W_pair, b_pair, W_type, b_type, **_unused):
    from concourse.bass_utils import run_bass_kernel_spmd

    assert int(n_fields) == F
    consts = host_constants(
        np.asarray(W_msr), np.asarray(b_msr), np.asarray(W_agg), np.asarray(b_agg),
        np.asarray(W_dim), np.asarray(b_dim), np.asarray(W_msrs), np.asarray(b_msrs),
        np.asarray(W_key), np.asarray(b_key), np.asarray(W_pair), np.asarray(b_pair),
        np.asarray(W_type), np.asarray(b_type),
    )
    if "nc" not in _NC_CACHE:
        _NC_CACHE["nc"] = build_program()
    nc = _NC_CACHE["nc"]
    in_maps = make_in_maps(
        np.asarray(tapas_embedding), np.asarray(col_ids), np.asarray(msr_pair_idx), consts
    )
    res = run_bass_kernel_spmd(nc, in_maps, list(range(NCORES))).results
    return assemble_outputs([res[i]["o_all"] for i in range(NCORES)])


# revision 34
# speedup vs baseline: 1.1072x; 1.0581x over previous
"""Trainium2 Bass kernel for nn_MetadataTapas (segment_reduce).

Strategy (pure data-parallel over batch, 4 rows per core on 8 cores):
  - segment-mean as a one-hot matmul on the TensorEngine:
      sums[f, h] = sum_s (col_ids[s] == f+1) * emb[s, h]
  - All seven heads are linear before log_softmax, so:
      head_out = diag(1/cnt) @ OneHot^T @ Emb @ W + b
    fe = sums * inv_cnt, transposed on the PE, then one fused matmul
    against the concatenated head weights (28 cols incl. the pair
    head's two 768-col halves u1/u2).
  - Pair gathers become selection matmuls: SelT[f, p] = (idx[p] == f)
    built on the DVE from a PE rank-1 broadcast, used as matmul weights
    against u = [u1 | u2].
  - log_softmax per head: reduce_max(negate) -> ACT exp(bias=-m,
    accum_out=sum) -> ACT ln -> fused tensor_scalar (x + (-m)) - ls.
  - Every DMA is contiguous: emb is one 3MB DMA per row, the small
    per-row inputs are host-packed into two tiny tensors, and all
    outputs leave as one [28, 128] block per row that the host
    reslices into the seven reference outputs.
"""

import os
import numpy as np
from contextlib import ExitStack

import concourse.bass as bass
import concourse.bacc as bacc
import concourse.mybir as mybir
import concourse.tile as tile

B, S, H, F, P, NTYPE = 32, 1024, 768, 128, 256, 7
NCORES = 8
RPC = B // NCORES          # batch rows per core
NCHUNK = S // 128          # 8 token chunks per row
KH = H // 128              # 6 contraction tiles over H
NH = 28                    # packed head cols: msr2 dim2 msrs2 key2 agg9 type7 u1_2 u2_2
METAW = 12                 # per-row meta cols: cid_f(8) inv(1) pad(3)

F32 = mybir.dt.float32
F32R = mybir.dt.float32r
BF16 = mybir.dt.bfloat16
I32 = mybir.dt.int32

HEAD_SLICES = [(0, 2), (2, 4), (4, 6), (6, 8), (8, 17), (17, 24)]

# Big matmuls in float32r: full-rate fp32 on the PE for moving dim >= 256.
# The BIR verifier requires fp32r matmul operands to be *produced* as
# float32r, so the whole emb/one-hot/idx path is typed float32r.
SEG_MM_F32R = os.environ.get("SEG_MM_F32R", "1") == "1"
EMB_DT = F32R if SEG_MM_F32R else F32


def _softmax_pre(nc, ps_ap, e_ap, negm_ap, ssum_ap):
    """reduce_max(negate) then exp(x - m) with accumulated sum.

    All Exp ops are batched before the single Ln per row so the ACT
    engine loads each activation table at most twice per row (table
    reloads were the dominant cost when Exp/Ln alternated)."""
    nc.vector.tensor_reduce(
        negm_ap, ps_ap, axis=mybir.AxisListType.X, op=mybir.AluOpType.max, negate=True
    )
    nc.scalar.activation(
        e_ap, ps_ap, mybir.ActivationFunctionType.Exp,
        bias=negm_ap, scale=1.0, accum_out=ssum_ap,
    )


def _build_body(ctx, tc, aps):
    nc = tc.nc
    emb, meta, pidxf = aps["emb"], aps["meta"], aps["pidxf"]
    o_all = aps["o_all"]

    const = ctx.enter_context(tc.tile_pool(name="const", bufs=1))
    embp = ctx.enter_context(tc.tile_pool(name="embp", bufs=4 * RPC))
    ohp = ctx.enter_context(tc.tile_pool(name="ohp", bufs=RPC))
    rowp = ctx.enter_context(tc.tile_pool(name="rowp", bufs=2))
    prep = ctx.enter_context(tc.tile_pool(name="prep", bufs=RPC))
    stat = ctx.enter_context(tc.tile_pool(name="stat", bufs=4))
    psA = ctx.enter_context(tc.tile_pool(name="psA", bufs=2, space="PSUM"))
    psT = ctx.enter_context(tc.tile_pool(name="psT", bufs=2, space="PSUM"))
    psS = ctx.enter_context(tc.tile_pool(name="psS", bufs=2, space="PSUM"))

    # all emb DMAs first: the sync HWDGE queue streams the 12.6MB
    # continuously from t=0 while small loads ride the scalar queue
    all_ets = []
    for r in range(RPC):
        for g in range(4):
            et = embp.tile([128, 2 * H], EMB_DT, tag="et")
            nc.sync.dma_start(
                et[:],
                emb[r, 2 * g * 128 : 2 * (g + 1) * 128, :].rearrange(
                    "(c p) h -> p c h", p=128
                ),
            )
            all_ets.append(et)

    # constants
    w_t = const.tile([128, KH * NH], EMB_DT, tag="w_t")
    nc.scalar.dma_start(w_t[:], aps["w_all"])
    b_t = const.tile([1, NH], F32, tag="b_t")
    nc.scalar.dma_start(b_t[:], aps["b_row"])
    ir_t = const.tile([128, 128], F32, tag="ir_t")
    nc.scalar.dma_start(ir_t[:], aps["iota_row"])
    ic_t = const.tile([128, 1], F32, tag="ic_t")
    nc.scalar.dma_start(ic_t[:], aps["iota_col"])
    id_t = const.tile([128, 128], F32, tag="id_t")
    nc.scalar.dma_start(id_t[:], aps["ident"])
    idr_t = const.tile([128, 128], EMB_DT, tag="idr_t")
    nc.scalar.dma_start(idr_t[:], aps["identr"])
    id2_t = const.tile([2, 2], F32, tag="id2_t")
    nc.scalar.dma_start(id2_t[:], aps["ident2"])
    on_t = const.tile([1, 128], F32, tag="on_t")
    nc.scalar.dma_start(on_t[:], aps["ones128"])
    onr_t = const.tile([1, 128], EMB_DT, tag="onr_t")
    nc.scalar.dma_start(onr_t[:], aps["ones128r"])

    # --- prologue: per-row small inputs, one-hots and pair selectors for ALL
    # rows up front, so the DVE never gates the next row's seg matmuls
    mts, oh_rows, selTs = [], [], []
    for r in range(RPC):
        mt = prep.tile([128, METAW], F32, tag="mt")
        nc.scalar.dma_start(mt[:], meta[r])
        mts.append(mt)

        pxf = prep.tile([1, 2 * P], EMB_DT, tag="pxf")
        nc.scalar.dma_start(pxf[:], pidxf[r])

        # broadcast idx row to all partitions via rank-1 matmul: ones^T @ idx
        idxb = psS.tile([128, 2 * P], F32, tag="sps")
        nc.tensor.matmul(idxb[:], onr_t[:], pxf[:], start=True, stop=True)
        selT = prep.tile([128, 2 * P], EMB_DT, tag="selT")
        nc.vector.tensor_scalar(
            out=selT[:], in0=idxb[:], scalar1=ic_t[:, 0:1], scalar2=None,
            op0=mybir.AluOpType.is_equal,
        )
        selTs.append(selT)

        # all 8 one-hot blocks in one DVE op via step-0 broadcasts:
        # oh_row[p, c, j] = (cid[p, c] == j + 1); bf16 is exact for 0/1 and
        # enables the PE fast-weight-load path
        oh_row = ohp.tile([128, NCHUNK * 128], EMB_DT, tag="oh")
        nc.vector.tensor_tensor(
            out=oh_row[:].rearrange("p (c j) -> p c j", c=NCHUNK),
            in0=mt[:, 0:NCHUNK].unsqueeze(2).broadcast_to([128, NCHUNK, 128]),
            in1=ir_t[:].unsqueeze(1).broadcast_to([128, NCHUNK, 128]),
            op=mybir.AluOpType.is_equal,
        )
        oh_rows.append(oh_row)

    for r in range(RPC):
        inv = mts[r][:, NCHUNK : NCHUNK + 1]
        oh_row = oh_rows[r]
        selT = selTs[r]

        ets = all_ets[4 * r : 4 * r + 4]

        # --- segment sums via one-hot matmul
        ps = psA.tile([128, 768], F32, tag="ps")
        for c in range(NCHUNK):
            oh = oh_row[:, bass.ts(c, 128)]
            et = ets[c // 2]
            off = (c % 2) * H
            nc.tensor.matmul(ps[:, 0:512], oh, et[:, off : off + 512],
                             start=(c == 0), stop=(c == NCHUNK - 1))
            nc.tensor.matmul(ps[:, 512:768], oh, et[:, off + 512 : off + 768],
                             start=(c == 0), stop=(c == NCHUNK - 1))

        # --- field embedding = sums * (1/max(cnt,1))  (inv_cnt host-derived)
        fe = rowp.tile([128, 768], EMB_DT, tag="fe")
        nc.vector.tensor_scalar_mul(fe[:], ps[:, 0:768], inv)

        # --- transpose fe -> feT (h on partitions)
        feT = rowp.tile([128, 768], EMB_DT, tag="feT")
        for k in range(KH):
            pt = psT.tile([128, 128], EMB_DT, tag="pt")
            nc.tensor.transpose(pt[:], fe[:, bass.ts(k, 128)], idr_t[:])
            nc.vector.tensor_copy(feT[:, bass.ts(k, 128)], pt[:])

        # --- all heads in one accumulated matmul; bias via rank-1 matmul
        ph = psS.tile([128, NH], F32, tag="sps")
        for k in range(KH):
            nc.tensor.matmul(
                ph[:], feT[:, bass.ts(k, 128)], w_t[:, bass.ts(k, NH)],
                start=(k == 0), stop=False,
            )
        nc.tensor.matmul(ph[:], on_t[:], b_t[:], start=False, stop=True)

        # pair-head rhs must live in SBUF
        u = rowp.tile([128, 4], EMB_DT, tag="u")
        nc.vector.tensor_copy(u[:], ph[:, 24:28])

        # --- log_softmax, batched: 8x (max, exp+accum), 1x ln, 8x fused sub
        negms = stat.tile([128, 8], F32, tag="negms")
        ssums = stat.tile([128, 8], F32, tag="ssums")
        lss = stat.tile([128, 8], F32, tag="lss")
        e_all = stat.tile([128, NH], F32, tag="e_all")
        ho = rowp.tile([128, NH], F32, tag="ho")
        for i, (a, b) in enumerate(HEAD_SLICES):
            _softmax_pre(nc, ph[:, a:b], e_all[:, a:b],
                         negms[:, i : i + 1], ssums[:, i : i + 1])

        # --- pair head: ppT[m, p] = u1[idx0[p], m] + u2[idx1[p], m]
        # (u as 2-col weights -> near-free LDWEIGHTS; N=256 f32r full rate)
        ppT = psS.tile([2, 2 * P], F32, tag="sps")
        nc.tensor.matmul(ppT[:, 0:P], u[:, 0:2], selT[:, 0:P], start=True, stop=False)
        nc.tensor.matmul(ppT[:, 0:P], u[:, 2:4], selT[:, P : 2 * P], start=False, stop=True)
        ppT_sb = rowp.tile([2, 2 * P], F32, tag="ppTs")
        nc.vector.tensor_copy(ppT_sb[:, 0:P], ppT[:, 0:P])
        pp2s = []
        for h in range(2):
            pp2 = psT.tile([128, 2], F32, tag="pt")
            nc.tensor.transpose(pp2[:], ppT_sb[:, bass.ts(h, 128)], id2_t[:])
            pp2s.append(pp2)
            _softmax_pre(nc, pp2[:, 0:2], e_all[:, 24 + 2 * h : 26 + 2 * h],
                         negms[:, 6 + h : 7 + h], ssums[:, 6 + h : 7 + h])

        nc.scalar.activation(lss[:], ssums[:], mybir.ActivationFunctionType.Ln)
        for i, (a, b) in enumerate(HEAD_SLICES):
            nc.vector.tensor_scalar(
                out=ho[:, a:b], in0=ph[:, a:b],
                scalar1=negms[:, i : i + 1], scalar2=lss[:, i : i + 1],
                op0=mybir.AluOpType.add, op1=mybir.AluOpType.subtract,
            )
        for h in range(2):
            nc.vector.tensor_scalar(
                out=ho[:, 24 + 2 * h : 26 + 2 * h], in0=pp2s[h][:, 0:2],
                scalar1=negms[:, 6 + h : 7 + h], scalar2=lss[:, 6 + h : 7 + h],
                op0=mybir.AluOpType.add, op1=mybir.AluOpType.subtract,
            )

        # --- transpose outputs and ship one contiguous block per row
        po = psT.tile([NH, 128], F32, tag="pt")
        nc.tensor.transpose(po[:], ho[:], id_t[:])
        oT = rowp.tile([NH, 128], F32, tag="oT")
        nc.vector.tensor_copy(oT[:], po[:])
        nc.scalar.dma_start(o_all[r], oT[:])


def build_program():
    nc = bacc.Bacc(trn_type="TRN2", target_bir_lowering=False, debug=False)
    aps = {}
    aps["emb"] = nc.dram_tensor("emb", [RPC, S, H], EMB_DT, kind="ExternalInput").ap()
    aps["meta"] = nc.dram_tensor("meta", [RPC, 128, METAW], F32, kind="ExternalInput").ap()
    aps["pidxf"] = nc.dram_tensor("pidxf", [RPC, 1, 2 * P], EMB_DT, kind="ExternalInput").ap()
    aps["w_all"] = nc.dram_tensor("w_all", [128, KH * NH], EMB_DT, kind="ExternalInput").ap()
    aps["b_row"] = nc.dram_tensor("b_row", [1, NH], F32, kind="ExternalInput").ap()
    aps["iota_row"] = nc.dram_tensor("iota_row", [128, 128], F32, kind="ExternalInput").ap()
    aps["iota_col"] = nc.dram_tensor("iota_col", [128, 1], F32, kind="ExternalInput").ap()
    aps["ident"] = nc.dram_tensor("ident", [128, 128], F32, kind="ExternalInput").ap()
    aps["identr"] = nc.dram_tensor("identr", [128, 128], EMB_DT, kind="ExternalInput").ap()
    aps["ident2"] = nc.dram_tensor("ident2", [2, 2], F32, kind="ExternalInput").ap()
    aps["ones128"] = nc.dram_tensor("ones128", [1, 128], F32, kind="ExternalInput").ap()
    aps["ones128r"] = nc.dram_tensor("ones128r", [1, 128], EMB_DT, kind="ExternalInput").ap()
    aps["o_all"] = nc.dram_tensor("o_all", [RPC, NH, 128], F32, kind="ExternalOutput").ap()

    with tile.TileContext(nc) as tc:
        with ExitStack() as ctx:
            _build_body(ctx, tc, aps)
    nc.compile()
    return nc


def host_constants(W_msr, b_msr, W_agg, b_agg, W_dim, b_dim, W_msrs, b_msrs,
                   W_key, b_key, W_pair, b_pair, W_type, b_type):
    f = np.float32
    W_all = np.concatenate(
        [W_msr, W_dim, W_msrs, W_key, W_agg, W_type, W_pair[:H], W_pair[H:]], axis=1
    ).astype(f)  # (768, 28)
    w_packed = np.ascontiguousarray(
        W_all.reshape(KH, 128, NH).transpose(1, 0, 2).reshape(128, KH * NH)
    )
    b_all = np.concatenate(
        [b_msr, b_dim, b_msrs, b_key, b_agg, b_type, b_pair, np.zeros(2, f)]
    ).astype(f).reshape(1, NH)
    return {
        "w_all": w_packed,
        "b_row": np.ascontiguousarray(b_all),
        "iota_row": np.tile(np.arange(1, 129, dtype=f), (128, 1)),
        "iota_col": np.arange(128, dtype=f).reshape(128, 1),
        "ident": np.eye(128, dtype=f),
        "identr": np.eye(128, dtype=f),
        "ident2": np.eye(2, dtype=f),
        "ones128": np.ones((1, 128), dtype=f),
        "ones128r": np.ones((1, 128), dtype=f),
    }


def make_in_maps(tapas_embedding, col_ids, msr_pair_idx, consts):
    f = np.float32
    in_maps = []
    for i in range(NCORES):
        sl = slice(i * RPC, (i + 1) * RPC)
        m = dict(consts)
        m["emb"] = np.ascontiguousarray(tapas_embedding[sl], dtype=f)
        cid = np.asarray(col_ids[sl], dtype=np.int64)
        # meta[r] = [cid as f32 (p, c) | 1/max(cnt,1) | pad]
        meta = np.zeros((RPC, 128, METAW), f)
        meta[:, :, 0:NCHUNK] = (
            cid.reshape(RPC, NCHUNK, 128).transpose(0, 2, 1).astype(f)
        )
        for r in range(RPC):
            cnt = np.bincount(cid[r], minlength=F + 1)
            meta[r, :, NCHUNK] = (1.0 / np.maximum(cnt[1:], 1)).astype(f)
        m["meta"] = meta
        # pair indices, j-major, as f32
        m["pidxf"] = np.ascontiguousarray(
            np.asarray(msr_pair_idx[sl], np.int64).transpose(0, 2, 1)
            .reshape(RPC, 1, 2 * P).astype(f)
        )
        in_maps.append(m)
    return in_maps


def assemble_outputs(o_all_list):
    """o_all per core: (RPC, 28, 128) -> the seven reference outputs."""
    o = np.concatenate([np.asarray(x) for x in o_all_list], 0)  # (n, 28, 128)
    n = o.shape[0]

    def head(a, b):
        return np.ascontiguousarray(o[:, a:b, :].transpose(0, 2, 1))

    pair = np.ascontiguousarray(
        o[:, 24:28, :].reshape(n, 2, 2, 128).transpose(0, 1, 3, 2).reshape(n, P, 2)
    )
    return (head(0, 2), head(8, 17), head(4, 6), head(2, 4), head(6, 8),
            pair, head(17, 24))


_NC_CACHE = {}


def kernel(tapas_embedding, col_ids, msr_pair_idx, n_fields,
           W_msr, b_msr, W_agg, b_agg, W_dim, b_dim, W_msrs, b_msrs,
           W_key, b_key, W_pair, b_pair, W_type, b_type, **_unused):
    from concourse.bass_utils import run_bass_kernel_spmd

    assert int(n_fields) == F
    consts = host_constants(
        np.asarray(W_msr), np.asarray(b_msr), np.asarray(W_agg), np.asarray(b_agg),
        np.asarray(W_dim), np.asarray(b_dim), np.asarray(W_msrs), np.asarray(b_msrs),
        np.asarray(W_key), np.asarray(b_key), np.asarray(W_pair), np.asarray(b_pair),
        np.asarray(W_type), np.asarray(b_type),
    )
    if "nc" not in _NC_CACHE:
        _NC_CACHE["nc"] = build_program()
    nc = _NC_CACHE["nc"]
    in_maps = make_in_maps(
        np.asarray(tapas_embedding), np.asarray(col_ids), np.asarray(msr_pair_idx), consts
    )
    res = run_bass_kernel_spmd(nc, in_maps, list(range(NCORES))).results
    return assemble_outputs([res[i]["o_all"] for i in range(NCORES)])


# revision 35
# speedup vs baseline: 1.1807x; 1.0663x over previous
"""Trainium2 Bass kernel for nn_MetadataTapas (segment_reduce).

Strategy (pure data-parallel over batch, 4 rows per core on 8 cores):
  - segment-mean as a one-hot matmul on the TensorEngine:
      sums[f, h] = sum_s (col_ids[s] == f+1) * emb[s, h]
  - All seven heads are linear before log_softmax, so:
      head_out = diag(1/cnt) @ OneHot^T @ Emb @ W + b
    fe = sums * inv_cnt, transposed on the PE, then one fused matmul
    against the concatenated head weights (28 cols incl. the pair
    head's two 768-col halves u1/u2).
  - Pair gathers become selection matmuls: SelT[f, p] = (idx[p] == f)
    built on the DVE from a PE rank-1 broadcast, used as matmul weights
    against u = [u1 | u2].
  - log_softmax per head: reduce_max(negate) -> ACT exp(bias=-m,
    accum_out=sum) -> ACT ln -> fused tensor_scalar (x + (-m)) - ls.
  - Every DMA is contiguous: emb is one 3MB DMA per row, the small
    per-row inputs are host-packed into two tiny tensors, and all
    outputs leave as one [28, 128] block per row that the host
    reslices into the seven reference outputs.
"""

import os
import numpy as np
from contextlib import ExitStack

import concourse.bass as bass
import concourse.bacc as bacc
import concourse.mybir as mybir
import concourse.tile as tile

B, S, H, F, P, NTYPE = 32, 1024, 768, 128, 256, 7
NCORES = 8
RPC = B // NCORES          # batch rows per core
NCHUNK = S // 128          # 8 token chunks per row
KH = H // 128              # 6 contraction tiles over H
NH = 28                    # packed head cols: msr2 dim2 msrs2 key2 agg9 type7 u1_2 u2_2
METAW = 12                 # per-row meta cols: cid_f(8) inv(1) pad(3)

F32 = mybir.dt.float32
F32R = mybir.dt.float32r
BF16 = mybir.dt.bfloat16
I32 = mybir.dt.int32

HEAD_SLICES = [(0, 2), (2, 4), (4, 6), (6, 8), (8, 17), (17, 24)]

# Big matmuls in float32r: full-rate fp32 on the PE for moving dim >= 256.
# The BIR verifier requires fp32r matmul operands to be *produced* as
# float32r, so the whole emb/one-hot/idx path is typed float32r.
SEG_MM_F32R = os.environ.get("SEG_MM_F32R", "1") == "1"
EMB_DT = F32R if SEG_MM_F32R else F32


def _softmax_pre(nc, ps_ap, e_ap, negm_ap, ssum_ap):
    """reduce_max(negate) then exp(x - m) with accumulated sum.

    All Exp ops are batched before the single Ln per row so the ACT
    engine loads each activation table at most twice per row (table
    reloads were the dominant cost when Exp/Ln alternated)."""
    nc.vector.tensor_reduce(
        negm_ap, ps_ap, axis=mybir.AxisListType.X, op=mybir.AluOpType.max, negate=True
    )
    nc.scalar.activation(
        e_ap, ps_ap, mybir.ActivationFunctionType.Exp,
        bias=negm_ap, scale=1.0, accum_out=ssum_ap,
    )


def _build_body(ctx, tc, aps):
    nc = tc.nc
    emb, meta, pidxf = aps["emb"], aps["meta"], aps["pidxf"]
    o_all = aps["o_all"]

    const = ctx.enter_context(tc.tile_pool(name="const", bufs=1))
    embp = ctx.enter_context(tc.tile_pool(name="embp", bufs=4 * RPC))
    ohp = ctx.enter_context(tc.tile_pool(name="ohp", bufs=RPC))
    rowp = ctx.enter_context(tc.tile_pool(name="rowp", bufs=2))
    prep = ctx.enter_context(tc.tile_pool(name="prep", bufs=RPC))
    stat = ctx.enter_context(tc.tile_pool(name="stat", bufs=4))
    psA = ctx.enter_context(tc.tile_pool(name="psA", bufs=2, space="PSUM"))
    psT = ctx.enter_context(tc.tile_pool(name="psT", bufs=2, space="PSUM"))
    psS = ctx.enter_context(tc.tile_pool(name="psS", bufs=1, space="PSUM"))
    psI = ctx.enter_context(tc.tile_pool(name="psI", bufs=1, space="PSUM"))

    # all emb DMAs first: the sync HWDGE queue streams the 12.6MB
    # continuously from t=0 while small loads ride the scalar queue
    all_ets = []
    for r in range(RPC):
        for g in range(4):
            et = embp.tile([128, 2 * H], EMB_DT, tag="et")
            nc.sync.dma_start(
                et[:],
                emb[r, 2 * g * 128 : 2 * (g + 1) * 128, :].rearrange(
                    "(c p) h -> p c h", p=128
                ),
            )
            all_ets.append(et)

    # critical-path constants + one-hots first
    ir_t = const.tile([128, 128], F32, tag="ir_t")
    nc.scalar.dma_start(ir_t[:], aps["iota_row"])
    mts, oh_rows, selTs = [], [], []
    for r in range(RPC):
        mt = prep.tile([128, METAW], F32, tag="mt")
        nc.scalar.dma_start(mt[:], meta[r])
        mts.append(mt)
    for r in range(RPC):
        # all 8 one-hot blocks in one DVE op via step-0 broadcasts:
        # oh_row[p, c, j] = (cid[p, c] == j + 1)
        oh_row = ohp.tile([128, NCHUNK * 128], EMB_DT, tag="oh")
        nc.vector.tensor_tensor(
            out=oh_row[:].rearrange("p (c j) -> p c j", c=NCHUNK),
            in0=mts[r][:, 0:NCHUNK].unsqueeze(2).broadcast_to([128, NCHUNK, 128]),
            in1=ir_t[:].unsqueeze(1).broadcast_to([128, NCHUNK, 128]),
            op=mybir.AluOpType.is_equal,
        )
        oh_rows.append(oh_row)

    # pair-selector prologue
    ic_t = const.tile([128, 1], F32, tag="ic_t")
    nc.scalar.dma_start(ic_t[:], aps["iota_col"])
    onr_t = const.tile([1, 128], EMB_DT, tag="onr_t")
    nc.scalar.dma_start(onr_t[:], aps["ones128r"])
    for r in range(RPC):
        pxf = prep.tile([1, 2 * P], EMB_DT, tag="pxf")
        nc.scalar.dma_start(pxf[:], pidxf[r])
        # broadcast idx row to all partitions via rank-1 matmul: ones^T @ idx
        idxb = psI.tile([128, 2 * P], F32, tag="idxb")
        nc.tensor.matmul(idxb[:], onr_t[:], pxf[:], start=True, stop=True)
        selT = prep.tile([128, 2 * P], EMB_DT, tag="selT")
        nc.vector.tensor_scalar(
            out=selT[:], in0=idxb[:], scalar1=ic_t[:, 0:1], scalar2=None,
            op0=mybir.AluOpType.is_equal,
        )
        selTs.append(selT)

    # remaining constants
    w_t = const.tile([128, KH * NH], EMB_DT, tag="w_t")
    nc.scalar.dma_start(w_t[:], aps["w_all"])
    b_t = const.tile([1, NH], F32, tag="b_t")
    nc.scalar.dma_start(b_t[:], aps["b_row"])
    id_t = const.tile([128, 128], F32, tag="id_t")
    nc.scalar.dma_start(id_t[:], aps["ident"])
    idr_t = const.tile([128, 128], EMB_DT, tag="idr_t")
    nc.scalar.dma_start(idr_t[:], aps["identr"])
    id2_t = const.tile([2, 2], F32, tag="id2_t")
    nc.scalar.dma_start(id2_t[:], aps["ident2"])
    on_t = const.tile([1, 128], F32, tag="on_t")
    nc.scalar.dma_start(on_t[:], aps["ones128"])

    for r in range(RPC):
        inv = mts[r][:, NCHUNK : NCHUNK + 1]
        oh_row = oh_rows[r]
        selT = selTs[r]

        ets = all_ets[4 * r : 4 * r + 4]

        # --- segment sums via one-hot matmul
        ps = psA.tile([128, 768], F32, tag="ps")
        for c in range(NCHUNK):
            oh = oh_row[:, bass.ts(c, 128)]
            et = ets[c // 2]
            off = (c % 2) * H
            nc.tensor.matmul(ps[:, 0:512], oh, et[:, off : off + 512],
                             start=(c == 0), stop=(c == NCHUNK - 1))
            nc.tensor.matmul(ps[:, 512:768], oh, et[:, off + 512 : off + 768],
                             start=(c == 0), stop=(c == NCHUNK - 1))

        # --- field embedding = sums * (1/max(cnt,1))  (inv_cnt host-derived)
        fe = rowp.tile([128, 768], EMB_DT, tag="fe")
        nc.vector.tensor_scalar_mul(fe[:], ps[:, 0:768], inv)

        # --- transpose fe -> feT (h on partitions)
        feT = rowp.tile([128, 768], EMB_DT, tag="feT")
        for k in range(KH):
            pt = psT.tile([128, 128], EMB_DT, tag="pt")
            nc.tensor.transpose(pt[:], fe[:, bass.ts(k, 128)], idr_t[:])
            nc.vector.tensor_copy(feT[:, bass.ts(k, 128)], pt[:])

        # --- all heads in one accumulated matmul; bias via rank-1 matmul
        ph = psS.tile([128, NH], F32, tag="sps")
        for k in range(KH):
            nc.tensor.matmul(
                ph[:], feT[:, bass.ts(k, 128)], w_t[:, bass.ts(k, NH)],
                start=(k == 0), stop=False,
            )
        nc.tensor.matmul(ph[:], on_t[:], b_t[:], start=False, stop=True)

        # pair-head rhs must live in SBUF
        u = rowp.tile([128, 4], EMB_DT, tag="u")
        nc.vector.tensor_copy(u[:], ph[:, 24:28])

        # --- log_softmax, batched: 8x (max, exp+accum), 1x ln, 8x fused sub
        negms = stat.tile([128, 8], F32, tag="negms")
        ssums = stat.tile([128, 8], F32, tag="ssums")
        lss = stat.tile([128, 8], F32, tag="lss")
        e_all = stat.tile([128, NH], F32, tag="e_all")
        ho = rowp.tile([128, NH], F32, tag="ho")
        for i, (a, b) in enumerate(HEAD_SLICES):
            _softmax_pre(nc, ph[:, a:b], e_all[:, a:b],
                         negms[:, i : i + 1], ssums[:, i : i + 1])

        # --- pair head: ppT[m, p] = u1[idx0[p], m] + u2[idx1[p], m]
        # (u as 2-col weights -> near-free LDWEIGHTS; N=256 f32r full rate)
        ppT = psT.tile([2, 2 * P], F32, tag="pt")
        nc.tensor.matmul(ppT[:, 0:P], u[:, 0:2], selT[:, 0:P], start=True, stop=False)
        nc.tensor.matmul(ppT[:, 0:P], u[:, 2:4], selT[:, P : 2 * P], start=False, stop=True)
        ppT_sb = rowp.tile([2, 2 * P], F32, tag="ppTs")
        nc.vector.tensor_copy(ppT_sb[:, 0:P], ppT[:, 0:P])
        pp2s = []
        for h in range(2):
            pp2 = psT.tile([128, 2], F32, tag="pt")
            nc.tensor.transpose(pp2[:], ppT_sb[:, bass.ts(h, 128)], id2_t[:])
            pp2s.append(pp2)
            _softmax_pre(nc, pp2[:, 0:2], e_all[:, 24 + 2 * h : 26 + 2 * h],
                         negms[:, 6 + h : 7 + h], ssums[:, 6 + h : 7 + h])

        nc.scalar.activation(lss[:], ssums[:], mybir.ActivationFunctionType.Ln)
        for i, (a, b) in enumerate(HEAD_SLICES):
            nc.vector.tensor_scalar(
                out=ho[:, a:b], in0=ph[:, a:b],
                scalar1=negms[:, i : i + 1], scalar2=lss[:, i : i + 1],
                op0=mybir.AluOpType.add, op1=mybir.AluOpType.subtract,
            )
        for h in range(2):
            nc.vector.tensor_scalar(
                out=ho[:, 24 + 2 * h : 26 + 2 * h], in0=pp2s[h][:, 0:2],
                scalar1=negms[:, 6 + h : 7 + h], scalar2=lss[:, 6 + h : 7 + h],
                op0=mybir.AluOpType.add, op1=mybir.AluOpType.subtract,
            )

        # --- transpose outputs and ship one contiguous block per row
        po = psT.tile([NH, 128], F32, tag="pt")
        nc.tensor.transpose(po[:], ho[:], id_t[:])
        oT = rowp.tile([NH, 128], F32, tag="oT")
        nc.vector.tensor_copy(oT[:], po[:])
        nc.scalar.dma_start(o_all[r], oT[:])


def build_program():
    nc = bacc.Bacc(trn_type="TRN2", target_bir_lowering=False, debug=False)
    aps = {}
    aps["emb"] = nc.dram_tensor("emb", [RPC, S, H], EMB_DT, kind="ExternalInput").ap()
    aps["meta"] = nc.dram_tensor("meta", [RPC, 128, METAW], F32, kind="ExternalInput").ap()
    aps["pidxf"] = nc.dram_tensor("pidxf", [RPC, 1, 2 * P], EMB_DT, kind="ExternalInput").ap()
    aps["w_all"] = nc.dram_tensor("w_all", [128, KH * NH], EMB_DT, kind="ExternalInput").ap()
    aps["b_row"] = nc.dram_tensor("b_row", [1, NH], F32, kind="ExternalInput").ap()
    aps["iota_row"] = nc.dram_tensor("iota_row", [128, 128], F32, kind="ExternalInput").ap()
    aps["iota_col"] = nc.dram_tensor("iota_col", [128, 1], F32, kind="ExternalInput").ap()
    aps["ident"] = nc.dram_tensor("ident", [128, 128], F32, kind="ExternalInput").ap()
    aps["identr"] = nc.dram_tensor("identr", [128, 128], EMB_DT, kind="ExternalInput").ap()
    aps["ident2"] = nc.dram_tensor("ident2", [2, 2], F32, kind="ExternalInput").ap()
    aps["ones128"] = nc.dram_tensor("ones128", [1, 128], F32, kind="ExternalInput").ap()
    aps["ones128r"] = nc.dram_tensor("ones128r", [1, 128], EMB_DT, kind="ExternalInput").ap()
    aps["o_all"] = nc.dram_tensor("o_all", [RPC, NH, 128], F32, kind="ExternalOutput").ap()

    with tile.TileContext(nc) as tc:
        with ExitStack() as ctx:
            _build_body(ctx, tc, aps)
    nc.compile()
    return nc


def host_constants(W_msr, b_msr, W_agg, b_agg, W_dim, b_dim, W_msrs, b_msrs,
                   W_key, b_key, W_pair, b_pair, W_type, b_type):
    f = np.float32
    W_all = np.concatenate(
        [W_msr, W_dim, W_msrs, W_key, W_agg, W_type, W_pair[:H], W_pair[H:]], axis=1
    ).astype(f)  # (768, 28)
    w_packed = np.ascontiguousarray(
        W_all.reshape(KH, 128, NH).transpose(1, 0, 2).reshape(128, KH * NH)
    )
    b_all = np.concatenate(
        [b_msr, b_dim, b_msrs, b_key, b_agg, b_type, b_pair, np.zeros(2, f)]
    ).astype(f).reshape(1, NH)
    return {
        "w_all": w_packed,
        "b_row": np.ascontiguousarray(b_all),
        "iota_row": np.tile(np.arange(1, 129, dtype=f), (128, 1)),
        "iota_col": np.arange(128, dtype=f).reshape(128, 1),
        "ident": np.eye(128, dtype=f),
        "identr": np.eye(128, dtype=f),
        "ident2": np.eye(2, dtype=f),
        "ones128": np.ones((1, 128), dtype=f),
        "ones128r": np.ones((1, 128), dtype=f),
    }


def make_in_maps(tapas_embedding, col_ids, msr_pair_idx, consts):
    f = np.float32
    in_maps = []
    for i in range(NCORES):
        sl = slice(i * RPC, (i + 1) * RPC)
        m = dict(consts)
        m["emb"] = np.ascontiguousarray(tapas_embedding[sl], dtype=f)
        cid = np.asarray(col_ids[sl], dtype=np.int64)
        # meta[r] = [cid as f32 (p, c) | 1/max(cnt,1) | pad]
        meta = np.zeros((RPC, 128, METAW), f)
        meta[:, :, 0:NCHUNK] = (
            cid.reshape(RPC, NCHUNK, 128).transpose(0, 2, 1).astype(f)
        )
        for r in range(RPC):
            cnt = np.bincount(cid[r], minlength=F + 1)
            meta[r, :, NCHUNK] = (1.0 / np.maximum(cnt[1:], 1)).astype(f)
        m["meta"] = meta
        # pair indices, j-major, as f32
        m["pidxf"] = np.ascontiguousarray(
            np.asarray(msr_pair_idx[sl], np.int64).transpose(0, 2, 1)
            .reshape(RPC, 1, 2 * P).astype(f)
        )
        in_maps.append(m)
    return in_maps


def assemble_outputs(o_all_list):
    """o_all per core: (RPC, 28, 128) -> the seven reference outputs."""
    o = np.concatenate([np.asarray(x) for x in o_all_list], 0)  # (n, 28, 128)
    n = o.shape[0]

    def head(a, b):
        return np.ascontiguousarray(o[:, a:b, :].transpose(0, 2, 1))

    pair = np.ascontiguousarray(
        o[:, 24:28, :].reshape(n, 2, 2, 128).transpose(0, 1, 3, 2).reshape(n, P, 2)
    )
    return (head(0, 2), head(8, 17), head(4, 6), head(2, 4), head(6, 8),
            pair, head(17, 24))


_NC_CACHE = {}


def kernel(tapas_embedding, col_ids, msr_pair_idx, n_fields,
           W_msr, b_msr, W_agg, b_agg, W_dim, b_dim, W_msrs, b_msrs,
           W_key, b_key, W_pair, b_pair, W_type, b_type, **_unused):
    from concourse.bass_utils import run_bass_kernel_spmd

    assert int(n_fields) == F
    consts = host_constants(
        np.asarray(W_msr), np.asarray(b_msr), np.asarray(W_agg), np.asarray(b_agg),
        np.asarray(W_dim), np.asarray(b_dim), np.asarray(W_msrs), np.asarray(b_msrs),
        np.asarray(W_key), np.asarray(b_key), np.asarray(W_pair), np.asarray(b_pair),
        np.asarray(W_type), np.asarray(b_type),
    )
    if "nc" not in _NC_CACHE:
        _NC_CACHE["nc"] = build_program()
    nc = _NC_CACHE["nc"]
    in_maps = make_in_maps(
        np.asarray(tapas_embedding), np.asarray(col_ids), np.asarray(msr_pair_idx), consts
    )
    res = run_bass_kernel_spmd(nc, in_maps, list(range(NCORES))).results
    return assemble_outputs([res[i]["o_all"] for i in range(NCORES)])


# revision 36
# speedup vs baseline: 1.2516x; 1.0601x over previous
"""Trainium2 Bass kernel for nn_MetadataTapas (segment_reduce).

Strategy (pure data-parallel over batch, 4 rows per core on 8 cores):
  - segment-mean as a one-hot matmul on the TensorEngine:
      sums[f, h] = sum_s (col_ids[s] == f+1) * emb[s, h]
  - All seven heads are linear before log_softmax, so:
      head_out = diag(1/cnt) @ OneHot^T @ Emb @ W + b
    fe = sums * inv_cnt, transposed on the PE, then one fused matmul
    against the concatenated head weights (28 cols incl. the pair
    head's two 768-col halves u1/u2).
  - Pair gathers become selection matmuls: SelT[f, p] = (idx[p] == f)
    built on the DVE from a PE rank-1 broadcast, used as matmul weights
    against u = [u1 | u2].
  - log_softmax per head: reduce_max(negate) -> ACT exp(bias=-m,
    accum_out=sum) -> ACT ln -> fused tensor_scalar (x + (-m)) - ls.
  - Every DMA is contiguous: emb is one 3MB DMA per row, the small
    per-row inputs are host-packed into two tiny tensors, and all
    outputs leave as one [28, 128] block per row that the host
    reslices into the seven reference outputs.
"""

import os
import numpy as np
from contextlib import ExitStack

import concourse.bass as bass
import concourse.bacc as bacc
import concourse.mybir as mybir
import concourse.tile as tile

B, S, H, F, P, NTYPE = 32, 1024, 768, 128, 256, 7
NCORES = 8
RPC = B // NCORES          # batch rows per core
NCHUNK = S // 128          # 8 token chunks per row
KH = H // 128              # 6 contraction tiles over H
NH = 28                    # packed head cols: msr2 dim2 msrs2 key2 agg9 type7 u1_2 u2_2
METAW = 12                 # per-row meta cols: cid_f(8) inv(1) pad(3)

F32 = mybir.dt.float32
F32R = mybir.dt.float32r
BF16 = mybir.dt.bfloat16
I32 = mybir.dt.int32

HEAD_SLICES = [(0, 2), (2, 4), (4, 6), (6, 8), (8, 17), (17, 24)]

# Big matmuls in float32r: full-rate fp32 on the PE for moving dim >= 256.
# The BIR verifier requires fp32r matmul operands to be *produced* as
# float32r, so the whole emb/one-hot/idx path is typed float32r.
SEG_MM_F32R = os.environ.get("SEG_MM_F32R", "1") == "1"
EMB_DT = F32R if SEG_MM_F32R else F32


def _softmax_pre(nc, ps_ap, e_ap, negm_ap, ssum_ap):
    """reduce_max(negate) then exp(x - m) with accumulated sum.

    All Exp ops are batched before the single Ln per row so the ACT
    engine loads each activation table at most twice per row (table
    reloads were the dominant cost when Exp/Ln alternated)."""
    nc.vector.tensor_reduce(
        negm_ap, ps_ap, axis=mybir.AxisListType.X, op=mybir.AluOpType.max, negate=True
    )
    nc.scalar.activation(
        e_ap, ps_ap, mybir.ActivationFunctionType.Exp,
        bias=negm_ap, scale=1.0, accum_out=ssum_ap,
    )


def _build_body(ctx, tc, aps):
    nc = tc.nc
    emb, meta, pidxf = aps["emb"], aps["meta"], aps["pidxf"]
    o_all = aps["o_all"]

    const = ctx.enter_context(tc.tile_pool(name="const", bufs=1))
    embp = ctx.enter_context(tc.tile_pool(name="embp", bufs=4 * RPC))
    ohp = ctx.enter_context(tc.tile_pool(name="ohp", bufs=RPC))
    rowp = ctx.enter_context(tc.tile_pool(name="rowp", bufs=2))
    prep = ctx.enter_context(tc.tile_pool(name="prep", bufs=RPC))
    stat = ctx.enter_context(tc.tile_pool(name="stat", bufs=4))
    psA = ctx.enter_context(tc.tile_pool(name="psA", bufs=1, space="PSUM"))
    psT = ctx.enter_context(tc.tile_pool(name="psT", bufs=2, space="PSUM"))
    psS = ctx.enter_context(tc.tile_pool(name="psS", bufs=2, space="PSUM"))
    psI = ctx.enter_context(tc.tile_pool(name="psI", bufs=1, space="PSUM"))

    # all emb DMAs first: the sync HWDGE queue streams the 12.6MB
    # continuously from t=0 while small loads ride the scalar queue
    all_ets = []
    for r in range(RPC):
        for g in range(4):
            et = embp.tile([128, 2 * H], EMB_DT, tag="et")
            nc.sync.dma_start(
                et[:],
                emb[r, 2 * g * 128 : 2 * (g + 1) * 128, :].rearrange(
                    "(c p) h -> p c h", p=128
                ),
            )
            all_ets.append(et)

    # critical-path constants + one-hots first
    ir_t = const.tile([128, 128], F32, tag="ir_t")
    nc.scalar.dma_start(ir_t[:], aps["iota_row"])
    mts, oh_rows, selTs = [], [], []
    for r in range(RPC):
        mt = prep.tile([128, METAW], F32, tag="mt")
        nc.scalar.dma_start(mt[:], meta[r])
        mts.append(mt)
    for r in range(RPC):
        # all 8 one-hot blocks in one DVE op via step-0 broadcasts:
        # oh_row[p, c, j] = (cid[p, c] == j + 1)
        oh_row = ohp.tile([128, NCHUNK * 128], EMB_DT, tag="oh")
        nc.vector.tensor_tensor(
            out=oh_row[:].rearrange("p (c j) -> p c j", c=NCHUNK),
            in0=mts[r][:, 0:NCHUNK].unsqueeze(2).broadcast_to([128, NCHUNK, 128]),
            in1=ir_t[:].unsqueeze(1).broadcast_to([128, NCHUNK, 128]),
            op=mybir.AluOpType.is_equal,
        )
        oh_rows.append(oh_row)

    # pair-selector prologue
    ic_t = const.tile([128, 1], F32, tag="ic_t")
    nc.scalar.dma_start(ic_t[:], aps["iota_col"])
    onr_t = const.tile([1, 128], EMB_DT, tag="onr_t")
    nc.scalar.dma_start(onr_t[:], aps["ones128r"])
    for r in range(RPC):
        pxf = prep.tile([1, 2 * P], EMB_DT, tag="pxf")
        nc.scalar.dma_start(pxf[:], pidxf[r])
        # broadcast idx row to all partitions via rank-1 matmul: ones^T @ idx
        idxb = psI.tile([128, 2 * P], F32, tag="idxb")
        nc.tensor.matmul(idxb[:], onr_t[:], pxf[:], start=True, stop=True)
        selT = prep.tile([128, 2 * P], EMB_DT, tag="selT")
        nc.vector.tensor_scalar(
            out=selT[:], in0=idxb[:], scalar1=ic_t[:, 0:1], scalar2=None,
            op0=mybir.AluOpType.is_equal,
        )
        selTs.append(selT)

    # remaining constants
    w_t = const.tile([128, KH * NH], EMB_DT, tag="w_t")
    nc.scalar.dma_start(w_t[:], aps["w_all"])
    b_t = const.tile([1, NH], F32, tag="b_t")
    nc.scalar.dma_start(b_t[:], aps["b_row"])
    id_t = const.tile([128, 128], F32, tag="id_t")
    nc.scalar.dma_start(id_t[:], aps["ident"])
    idr_t = const.tile([128, 128], EMB_DT, tag="idr_t")
    nc.scalar.dma_start(idr_t[:], aps["identr"])
    id2_t = const.tile([2, 2], F32, tag="id2_t")
    nc.scalar.dma_start(id2_t[:], aps["ident2"])
    on_t = const.tile([1, 128], F32, tag="on_t")
    nc.scalar.dma_start(on_t[:], aps["ones128"])

    def _row_front(r):
        """seg matmuls -> fe -> feT -> head matmul -> u  (PE-dense, no ACT deps)"""
        inv = mts[r][:, NCHUNK : NCHUNK + 1]
        oh_row = oh_rows[r]
        ets = all_ets[4 * r : 4 * r + 4]

        ps = psA.tile([128, 768], F32, tag="ps")
        for c in range(NCHUNK):
            oh = oh_row[:, bass.ts(c, 128)]
            et = ets[c // 2]
            off = (c % 2) * H
            nc.tensor.matmul(ps[:, 0:512], oh, et[:, off : off + 512],
                             start=(c == 0), stop=(c == NCHUNK - 1))
            nc.tensor.matmul(ps[:, 512:768], oh, et[:, off + 512 : off + 768],
                             start=(c == 0), stop=(c == NCHUNK - 1))

        # field embedding = sums * (1/max(cnt,1))  (inv_cnt host-derived)
        fe = rowp.tile([128, 768], EMB_DT, tag="fe")
        nc.vector.tensor_scalar_mul(fe[:], ps[:, 0:768], inv)

        # transpose fe -> feT (h on partitions)
        feT = rowp.tile([128, 768], EMB_DT, tag="feT")
        for k in range(KH):
            pt = psT.tile([128, 128], EMB_DT, tag="pt")
            nc.tensor.transpose(pt[:], fe[:, bass.ts(k, 128)], idr_t[:])
            nc.vector.tensor_copy(feT[:, bass.ts(k, 128)], pt[:])

        # all heads in one accumulated matmul; bias via rank-1 matmul
        ph = psS.tile([128, NH], F32, tag="sps")
        for k in range(KH):
            nc.tensor.matmul(
                ph[:], feT[:, bass.ts(k, 128)], w_t[:, bass.ts(k, NH)],
                start=(k == 0), stop=False,
            )
        nc.tensor.matmul(ph[:], on_t[:], b_t[:], start=False, stop=True)

        # pair-head rhs must live in SBUF
        u = rowp.tile([128, 4], EMB_DT, tag="u")
        nc.vector.tensor_copy(u[:], ph[:, 24:28])
        return ph, u

    def _row_back(r, ph, u):
        """softmax + pair gathers + transposed output block"""
        selT = selTs[r]
        negms = stat.tile([128, 8], F32, tag="negms")
        ssums = stat.tile([128, 8], F32, tag="ssums")
        lss = stat.tile([128, 8], F32, tag="lss")
        e_all = stat.tile([128, NH], F32, tag="e_all")
        ho = rowp.tile([128, NH], F32, tag="ho")
        for i, (a, b) in enumerate(HEAD_SLICES):
            _softmax_pre(nc, ph[:, a:b], e_all[:, a:b],
                         negms[:, i : i + 1], ssums[:, i : i + 1])

        # pair head: ppT[m, p] = u1[idx0[p], m] + u2[idx1[p], m]
        # (u as 2-col weights -> near-free LDWEIGHTS; N=256 f32r full rate)
        ppT = psT.tile([2, 2 * P], F32, tag="pt")
        nc.tensor.matmul(ppT[:, 0:P], u[:, 0:2], selT[:, 0:P], start=True, stop=False)
        nc.tensor.matmul(ppT[:, 0:P], u[:, 2:4], selT[:, P : 2 * P], start=False, stop=True)
        ppT_sb = rowp.tile([2, 2 * P], F32, tag="ppTs")
        nc.vector.tensor_copy(ppT_sb[:, 0:P], ppT[:, 0:P])
        pp2s = []
        for h in range(2):
            pp2 = psT.tile([128, 2], F32, tag="pt")
            nc.tensor.transpose(pp2[:], ppT_sb[:, bass.ts(h, 128)], id2_t[:])
            pp2s.append(pp2)
            _softmax_pre(nc, pp2[:, 0:2], e_all[:, 24 + 2 * h : 26 + 2 * h],
                         negms[:, 6 + h : 7 + h], ssums[:, 6 + h : 7 + h])

        nc.scalar.activation(lss[:], ssums[:], mybir.ActivationFunctionType.Ln)
        for i, (a, b) in enumerate(HEAD_SLICES):
            nc.vector.tensor_scalar(
                out=ho[:, a:b], in0=ph[:, a:b],
                scalar1=negms[:, i : i + 1], scalar2=lss[:, i : i + 1],
                op0=mybir.AluOpType.add, op1=mybir.AluOpType.subtract,
            )
        for h in range(2):
            nc.vector.tensor_scalar(
                out=ho[:, 24 + 2 * h : 26 + 2 * h], in0=pp2s[h][:, 0:2],
                scalar1=negms[:, 6 + h : 7 + h], scalar2=lss[:, 6 + h : 7 + h],
                op0=mybir.AluOpType.add, op1=mybir.AluOpType.subtract,
            )

        # transpose outputs and ship one contiguous block per row
        po = psT.tile([NH, 128], F32, tag="pt")
        nc.tensor.transpose(po[:], ho[:], id_t[:])
        oT = rowp.tile([NH, 128], F32, tag="oT")
        nc.vector.tensor_copy(oT[:], po[:])
        nc.sync.dma_start(o_all[r], oT[:])

    # 1-row software pipeline: row r's softmax/pair/output is emitted after
    # row r+1's PE-dense front, so the PE never waits on the ACT/DVE chain
    pend = None
    for r in range(RPC):
        cur = _row_front(r)
        if pend is not None:
            _row_back(r - 1, *pend)
        pend = cur
    _row_back(RPC - 1, *pend)


def build_program():
    nc = bacc.Bacc(trn_type="TRN2", target_bir_lowering=False, debug=False)
    aps = {}
    aps["emb"] = nc.dram_tensor("emb", [RPC, S, H], EMB_DT, kind="ExternalInput").ap()
    aps["meta"] = nc.dram_tensor("meta", [RPC, 128, METAW], F32, kind="ExternalInput").ap()
    aps["pidxf"] = nc.dram_tensor("pidxf", [RPC, 1, 2 * P], EMB_DT, kind="ExternalInput").ap()
    aps["w_all"] = nc.dram_tensor("w_all", [128, KH * NH], EMB_DT, kind="ExternalInput").ap()
    aps["b_row"] = nc.dram_tensor("b_row", [1, NH], F32, kind="ExternalInput").ap()
    aps["iota_row"] = nc.dram_tensor("iota_row", [128, 128], F32, kind="ExternalInput").ap()
    aps["iota_col"] = nc.dram_tensor("iota_col", [128, 1], F32, kind="ExternalInput").ap()
    aps["ident"] = nc.dram_tensor("ident", [128, 128], F32, kind="ExternalInput").ap()
    aps["identr"] = nc.dram_tensor("identr", [128, 128], EMB_DT, kind="ExternalInput").ap()
    aps["ident2"] = nc.dram_tensor("ident2", [2, 2], F32, kind="ExternalInput").ap()
    aps["ones128"] = nc.dram_tensor("ones128", [1, 128], F32, kind="ExternalInput").ap()
    aps["ones128r"] = nc.dram_tensor("ones128r", [1, 128], EMB_DT, kind="ExternalInput").ap()
    aps["o_all"] = nc.dram_tensor("o_all", [RPC, NH, 128], F32, kind="ExternalOutput").ap()

    with tile.TileContext(nc) as tc:
        with ExitStack() as ctx:
            _build_body(ctx, tc, aps)
    nc.compile()
    return nc


def host_constants(W_msr, b_msr, W_agg, b_agg, W_dim, b_dim, W_msrs, b_msrs,
                   W_key, b_key, W_pair, b_pair, W_type, b_type):
    f = np.float32
    W_all = np.concatenate(
        [W_msr, W_dim, W_msrs, W_key, W_agg, W_type, W_pair[:H], W_pair[H:]], axis=1
    ).astype(f)  # (768, 28)
    w_packed = np.ascontiguousarray(
        W_all.reshape(KH, 128, NH).transpose(1, 0, 2).reshape(128, KH * NH)
    )
    b_all = np.concatenate(
        [b_msr, b_dim, b_msrs, b_key, b_agg, b_type, b_pair, np.zeros(2, f)]
    ).astype(f).reshape(1, NH)
    return {
        "w_all": w_packed,
        "b_row": np.ascontiguousarray(b_all),
        "iota_row": np.tile(np.arange(1, 129, dtype=f), (128, 1)),
        "iota_col": np.arange(128, dtype=f).reshape(128, 1),
        "ident": np.eye(128, dtype=f),
        "identr": np.eye(128, dtype=f),
        "ident2": np.eye(2, dtype=f),
        "ones128": np.ones((1, 128), dtype=f),
        "ones128r": np.ones((1, 128), dtype=f),
    }


def make_in_maps(tapas_embedding, col_ids, msr_pair_idx, consts):
    f = np.float32
    in_maps = []
    for i in range(NCORES):
        sl = slice(i * RPC, (i + 1) * RPC)
        m = dict(consts)
        m["emb"] = np.ascontiguousarray(tapas_embedding[sl], dtype=f)
        cid = np.asarray(col_ids[sl], dtype=np.int64)
        # meta[r] = [cid as f32 (p, c) | 1/max(cnt,1) | pad]
        meta = np.zeros((RPC, 128, METAW), f)
        meta[:, :, 0:NCHUNK] = (
            cid.reshape(RPC, NCHUNK, 128).transpose(0, 2, 1).astype(f)
        )
        for r in range(RPC):
            cnt = np.bincount(cid[r], minlength=F + 1)
            meta[r, :, NCHUNK] = (1.0 / np.maximum(cnt[1:], 1)).astype(f)
        m["meta"] = meta
        # pair indices, j-major, as f32
        m["pidxf"] = np.ascontiguousarray(
            np.asarray(msr_pair_idx[sl], np.int64).transpose(0, 2, 1)
            .reshape(RPC, 1, 2 * P).astype(f)
        )
        in_maps.append(m)
    return in_maps


def assemble_outputs(o_all_list):
    """o_all per core: (RPC, 28, 128) -> the seven reference outputs."""
    o = np.concatenate([np.asarray(x) for x in o_all_list], 0)  # (n, 28, 128)
    n = o.shape[0]

    def head(a, b):
        return np.ascontiguousarray(o[:, a:b, :].transpose(0, 2, 1))

    pair = np.ascontiguousarray(
        o[:, 24:28, :].reshape(n, 2, 2, 128).transpose(0, 1, 3, 2).reshape(n, P, 2)
    )
    return (head(0, 2), head(8, 17), head(4, 6), head(2, 4), head(6, 8),
            pair, head(17, 24))


_NC_CACHE = {}


def kernel(tapas_embedding, col_ids, msr_pair_idx, n_fields,
           W_msr, b_msr, W_agg, b_agg, W_dim, b_dim, W_msrs, b_msrs,
           W_key, b_key, W_pair, b_pair, W_type, b_type, **_unused):
    from concourse.bass_utils import run_bass_kernel_spmd

    assert int(n_fields) == F
    consts = host_constants(
        np.asarray(W_msr), np.asarray(b_msr), np.asarray(W_agg), np.asarray(b_agg),
        np.asarray(W_dim), np.asarray(b_dim), np.asarray(W_msrs), np.asarray(b_msrs),
        np.asarray(W_key), np.asarray(b_key), np.asarray(W_pair), np.asarray(b_pair),
        np.asarray(W_type), np.asarray(b_type),
    )
    if "nc" not in _NC_CACHE:
        _NC_CACHE["nc"] = build_program()
    nc = _NC_CACHE["nc"]
    in_maps = make_in_maps(
        np.asarray(tapas_embedding), np.asarray(col_ids), np.asarray(msr_pair_idx), consts
    )
    res = run_bass_kernel_spmd(nc, in_maps, list(range(NCORES))).results
    return assemble_outputs([res[i]["o_all"] for i in range(NCORES)])
